# revision 17
# baseline (speedup 1.0000x reference)
"""Masked attention (B=2, H=8, S=4096, D=64) on 8 Trainium2 NeuronCores.

Sharding: batch*head parallel. Core c owns flat heads {2c, 2c+1} (same batch
index b = c // 4 for both, so the [S, S] mask is shared by both heads of a
core).

Device algorithm (per core, per head), transposed so no on-chip transposes are
ever needed:

  - Host supplies K^T augmented with a ones row as [65, S] fp16, Q^T pre-scaled
    by A*SCALE with a B row as [65, S] fp16, so the score matmul directly
    produces y[k, q] = A*x + B in PSUM, where x = (q . k)/sqrt(D) is the true
    logit, A = 1024/ln2 and B = 1024*(15 - c_rms). V is laid out chunk-major
    per partition as [128, n_chunks*64] fp16 so its DMA is contiguous. The
    mask is pre-tiled by the host into contiguous [128, 2048] DMA tiles (one
    per (q-block, pair-of-chunk-groups)), streamed on the GpSimd software-DGE
    queue so mask issue ops never serialize behind const/output DMAs on Sync.
  - Scores are computed transposed: y[k, q] via matmul(lhsT=K^T chunk [65,128],
    rhs=Q^T block [65, 512]); fp16 streams 1 column/cycle and keeps the HAM
    clock at 2.4 GHz.
  - Per score tile (a (2-chunk group, head) pair), one of two drain paths,
    statically scheduled to balance ScalarE/VectorE/GpSimd against the PE:
      ACT path: ScalarE activation computes pt = exp(y/A - B/A) = exp(x)
        (PSUM->SBUF fp16), then VectorE (or, for a gp_frac fraction, GpSimd)
        multiplies by the keep-mask (fp16 2x).
      DVE path: one fused VectorE tensor_mul with int16 output:
        i16 = convert(y * maskT). Bit-cast as fp16 this is Schraudolph's
        approximate exp (rel err ~1.7% RMS), and masked entries are exactly
        0x0000 = 0.0. One 1x-rate pass does drain+mask+exp, freeing ScalarE.
    No max-subtraction is needed: scores ~ N(0,1), exp stays in range.
  - AV accumulates transposed-free with M=64: matmul(lhsT=V chunk [128,64],
    rhs=P^T chunk [128,512], fp16) accumulates out^T[d,q] over the 32 k-chunks
    in PSUM. With only 64 output partitions the PE streams the moving operand
    at 2 columns/cycle (column-tiling mode), so AV costs half the score
    matmul. The softmax denominator is NOT computed on device: the host
    replicates the per-tile exp formulas (true exp for ACT tiles, the exact
    int16-Schraudolph for DVE tiles) from the same fp16-rounded Q/K and
    divides during unshard; per-element rounding deltas vs the device are
    ~1e-3 relative and average out over the ~2048 unmasked keys.
  - A finished AV accumulator is flushed (PSUM -> SBUF copy on alternating
    Scalar/Vector, then DMA to DRAM) immediately when its last chunk's AV
    matmul is emitted, so the copy clears the PSUM bank well before the next
    q-block's accumulation needs it.
"""

from contextlib import ExitStack

import numpy as np

import concourse.tile as tile
from concourse import bacc, mybir
from concourse.bass_utils import run_bass_kernel_spmd

B, H, S, D = 2, 8, 4096, 64
N_CORES = 8
HPC = (B * H) // N_CORES  # heads per core = 2
SCALE = 1.0 / 8.0  # 1/sqrt(D)

# Schraudolph constants for fp16 (10-bit mantissa, bias 15):
#   i16 = round(A*x + B); bitcast(i16) ~= exp(x), rel err ~1.7% RMS.
# A = 1024/ln2; B = 1024*(15 - c) with c ~= 0.0573 (RMS-optimal), rounded so
# B is exactly representable in fp16 (it is sent as a Q^T row).
A_CONST = 1477.3197218702985
B_CONST = 15304.0

F32 = mybir.dt.float32
BF16 = mybir.dt.bfloat16
F16 = mybir.dt.float16
I16 = mybir.dt.int16

# Drain-path schedule (must be identical between the device program and the
# host denominator): slot si = (qb*n_groups + gi)*hpc + h; DVE path iff
# si % DVE_PERIOD in DVE_SLOTS.
DVE_PERIOD = 36
DVE_SLOTS = (2, 5, 8, 10, 13, 16, 19, 21, 24, 27, 30, 32, 35)


def build_kernel_body(tc, qT, kT, vres, maskT, outT, s=S, hpc=HPC, qb_size=512,
                      group_size=2, psum_s_bufs=3, pt_bufs=8, mask_bufs=8,
                      dve_period=DVE_PERIOD, dve_slots=DVE_SLOTS,
                      gp_frac=10.0 / 23.0, mask_lookahead=3, av_phases=1,
                      warm_mms=12, pair=2):
    """Emit the attention program. All APs are DRAM tensors:
    qT, kT: [hpc, 65, s] f16; vres: [hpc, 128, n_chunks*64] f16;
    maskT: [n_qb, n_pairs, 128, pair*group_size*qb_size] f16 (pre-tiled);
    outT: [hpc, 64, s] f32 (unnormalized AV, host divides by denominator).
    """
    nc = tc.nc
    n_qb = s // qb_size
    n_chunks = s // 128
    groups = []
    c0 = 0
    while c0 < n_chunks:
        groups.append((c0, min(group_size, n_chunks - c0)))
        c0 += group_size
    n_pairs = len(groups) // pair

    ctx = ExitStack()
    const = ctx.enter_context(tc.tile_pool(name="const", bufs=1))
    mask_pool = ctx.enter_context(tc.tile_pool(name="mask", bufs=mask_bufs))
    pt_pool = ctx.enter_context(tc.tile_pool(name="pt", bufs=max(
        pt_bufs, (len(groups) // av_phases) * hpc + 4)))
    out_pool = ctx.enter_context(tc.tile_pool(name="osb", bufs=3))
    psum_s_pool = ctx.enter_context(
        tc.tile_pool(name="psum_s", bufs=psum_s_bufs, space="PSUM"))
    psum_av_pool = ctx.enter_context(
        tc.tile_pool(name="psum_av", bufs=hpc, space="PSUM"))

    # Resident tensors: Q^T, K^T (fp16, 65 rows: d + affine row), V chunked.
    qT_sb = const.tile([D + 1, hpc, s], F16)
    kT_sb = const.tile([D + 1, hpc, s], F16)
    vres_sb = const.tile([128, hpc, n_chunks, D], F16)
    # Per-partition bias for the ACT path: exp(y/A - B/A) = exp(x).
    bias_sb = const.tile([128, 1], F32)
    nc.gpsimd.memset(bias_sb[:, :], -B_CONST / A_CONST)
    # Dummy 1-element exp so the ~2.7us ACT_TABLE_LOAD happens during the
    # prologue DMA debt instead of delaying the first real exp.
    tl_sb = const.tile([128, 1], F32)
    nc.scalar.activation(tl_sb[:, :], bias_sb[:, :],
                         mybir.ActivationFunctionType.Exp)

    # Mask tiles stream on the GpSimd software-DGE queue as flat pre-tiled 2D
    # blocks (cheap descriptors; each DMA covers `pair` chunk-groups), so
    # their issue ops and buffer-reuse waits never serialize behind the big
    # const loads or the output DMAs on the Sync queue.
    mask_plan = [(qb_, pr_) for qb_ in range(n_qb) for pr_ in range(n_pairs)]
    mask_tiles = {}
    mask_next = [0]

    def issue_masks(upto_pairs):
        while mask_next[0] < min(upto_pairs, len(mask_plan)):
            qb_, pr_ = mask_plan[mask_next[0]]
            mt = mask_pool.tile([128, pair, group_size, qb_size], F16)
            nc.gpsimd.dma_start(
                out=mt[:, :, :, :],
                in_=maskT[qb_, pr_].rearrange(
                    "p (g c q) -> p g c q", g=pair, c=group_size),
            )
            for g2 in range(pair):
                mask_tiles[(qb_, pr_ * pair + g2)] = mt[:, g2]
            mask_next[0] += 1

    # Prologue DMAs. The Sync hardware-DGE queue barely delivers anything in
    # the first ~15us, while the GpSimd software-DGE ramps fast — so the
    # time-critical loads (first K^T/Q^T pieces and the early masks) go on
    # the software queue in need-order, and everything needed after ~20us
    # (late K^T pieces, Q^T remainder, V) goes on Sync.
    vres_r = [vres[h, :, :].rearrange("p (c w) -> p c w", w=D)
              for h in range(hpc)]
    for h in range(hpc):
        nc.gpsimd.dma_start(out=kT_sb[:, h, 0:qb_size],
                            in_=kT[h, :, 0:qb_size])
        nc.gpsimd.dma_start(out=qT_sb[:, h, 0:qb_size],
                            in_=qT[h, :, 0:qb_size])
    issue_masks(1)
    for h in range(hpc):
        nc.gpsimd.dma_start(out=kT_sb[:, h, qb_size:2 * qb_size],
                            in_=kT[h, :, qb_size:2 * qb_size])
    issue_masks(2)
    for h in range(hpc):
        nc.gpsimd.dma_start(out=kT_sb[:, h, 2 * qb_size:4 * qb_size],
                            in_=kT[h, :, 2 * qb_size:4 * qb_size])
    issue_masks(3)
    # Sync: later K^T pieces (needed from ~slot 16), then V (needed from the
    # first AV phase, ~34us in) and the Q^T remainder (needed from qb 1).
    for h in range(hpc):
        nc.sync.dma_start(out=kT_sb[:, h, 4 * qb_size:6 * qb_size],
                          in_=kT[h, :, 4 * qb_size:6 * qb_size])
    for h in range(hpc):
        nc.sync.dma_start(out=kT_sb[:, h, 6 * qb_size:],
                          in_=kT[h, :, 6 * qb_size:])
    for h in range(hpc):
        nc.sync.dma_start(out=vres_sb[:, h, 0:n_chunks // 2, :],
                          in_=vres_r[h][:, 0:n_chunks // 2, :])
    for h in range(hpc):
        nc.sync.dma_start(out=vres_sb[:, h, n_chunks // 2:, :],
                          in_=vres_r[h][:, n_chunks // 2:, :])
    for h in range(hpc):
        nc.sync.dma_start(out=qT_sb[:, h, qb_size:], in_=qT[h, :, qb_size:])

    # HAM warm-up: fp16 matmuls on a memset tile, needing no DMA — they span
    # the prologue DMA debt and bring the PE clock to 2.4 GHz before the
    # first real score matmul issues.
    warm = const.tile([128, qb_size], F16)
    nc.vector.memset(warm, 0.0)
    wp = psum_s_pool.tile([128, group_size, qb_size], F32, name="wp", tag="ps")
    for _ in range(warm_mms):
        nc.tensor.matmul(wp[:, 0, :], lhsT=warm[:, 0:128], rhs=warm[:, :],
                         start=True, stop=True)

    # Flat slot schedule: (qb, group, head). Score matmuls for a q-block are
    # emitted slot by slot (uniform 128-row tile config); the AV matmuls are
    # emitted in av_phases uniform M=64 bursts per q-block so the PE's
    # column-tiling configuration only changes 2*av_phases times per q-block
    # (mixing (128,128) and (128,64) matmuls per slot thrashes the PE
    # pipeline and HAM).
    slots = []
    for qb in range(n_qb):
        for gi, (c0_, gs_) in enumerate(groups):
            for h in range(hpc):
                slots.append((qb, gi, c0_, gs_, h))

    pt_store = {}
    flush_count = [0]
    av_cur = [None]
    phase_marks = {}
    gpp = len(groups) // av_phases
    for ph in range(av_phases):
        lo = ph * gpp
        hi = (ph + 1) * gpp if ph < av_phases - 1 else len(groups)
        phase_marks[hi - 1] = (lo, hi)

    def emit_av_phase(qb, g_lo, g_hi):
        if av_cur[0] is None:
            av_cur[0] = psum_av_pool.tile([128, qb_size], F32, tag="av",
                                          name="av")
        avt = av_cur[0]
        for gi in range(g_lo, g_hi):
            c0_, gs_ = groups[gi]
            for j in range(gs_):
                c = c0_ + j
                for h in range(hpc):
                    nc.tensor.matmul(
                        avt[h * D:(h + 1) * D, :],
                        lhsT=vres_sb[:, h, c, :],
                        rhs=pt_store[(gi, h)][:, j, :],
                        start=(c == 0),
                        stop=(c == n_chunks - 1),
                        tile_position=(0, h * D),
                    )
        if g_hi == len(groups):
            # Drain the finished accumulator (both heads): PSUM -> SBUF on
            # alternating engines, then one DMA to DRAM.
            osb = out_pool.tile([128, qb_size], F32, name="osb")
            if flush_count[0] % 2 == 0:
                nc.vector.tensor_copy(osb[:, :], avt[:, :])
            else:
                nc.scalar.copy(osb[:, :], avt[:, :])
            flush_count[0] += 1
            nc.sync.dma_start(out=outT[qb], in_=osb[:, :])
            av_cur[0] = None

    pending_tt = []
    gp_credit = [0.0]
    for si, (qb, gi, c0_, gs_, h) in enumerate(slots):
        qs = slice(qb * qb_size, (qb + 1) * qb_size)
        if h == 0:
            gidx = qb * len(groups) + gi
            issue_masks(gidx // pair + 1 + mask_lookahead)
        mt = mask_tiles[(qb, gi)]

        ps = psum_s_pool.tile([128, group_size, qb_size], F32)
        for j in range(gs_):
            c = c0_ + j
            nc.tensor.matmul(
                ps[:, j, :],
                lhsT=kT_sb[:, h, c * 128:(c + 1) * 128],
                rhs=qT_sb[:, h, qs],
                start=True,
                stop=True,
            )

        pt = pt_pool.tile([128, group_size, qb_size], F16)
        is_dve = (si % dve_period) in dve_slots
        if is_dve:
            # Fused drain+mask+exp on VectorE: i16 = convert(y*mask); the fp16
            # bit pattern of i16 = round(A*x+B) approximates exp(x); mask=0
            # gives exactly 0.0. Emitted ahead of the previous ACT slot's
            # mask-multiply so it runs concurrently with that ACTIVATE on the
            # in-order Vector queue.
            nc.vector.tensor_mul(
                pt[:, :gs_, :].bitcast(I16), ps[:, :gs_, :], mt[:, :gs_, :])
        else:
            nc.scalar.activation(
                pt[:, :gs_, :], ps[:, :gs_, :],
                mybir.ActivationFunctionType.Exp,
                scale=1.0 / A_CONST, bias=bias_sb[:, :],
            )
        # Previous ACT slot's mask-multiply: deferred one slot so this slot's
        # fused DVE drain (if any) sits ahead of it on the Vector queue.
        if pending_tt:
            opt, omt, ogs, use_gp = pending_tt.pop()
            eng = nc.gpsimd if use_gp else nc.vector
            eng.tensor_mul(opt[:, :ogs, :], opt[:, :ogs, :], omt[:, :ogs, :])
        if not is_dve:
            gp_credit[0] += gp_frac
            use_gp = gp_credit[0] >= 1.0
            if use_gp:
                gp_credit[0] -= 1.0
            pending_tt.append((pt, mt, gs_, use_gp))

        pt_store[(gi, h)] = pt
        if h == hpc - 1 and gi in phase_marks:
            # The AV matmuls read pt in place: any outstanding deferred
            # mask-multiply must be emitted first so Tile orders it before
            # the AV reads.
            if pending_tt:
                opt, omt, ogs, use_gp = pending_tt.pop()
                eng = nc.gpsimd if use_gp else nc.vector
                eng.tensor_mul(opt[:, :ogs, :], opt[:, :ogs, :],
                               omt[:, :ogs, :])
            emit_av_phase(qb, *phase_marks[gi])
    if pending_tt:
        opt, omt, ogs, use_gp = pending_tt.pop()
        eng = nc.gpsimd if use_gp else nc.vector
        eng.tensor_mul(opt[:, :ogs, :], opt[:, :ogs, :], omt[:, :ogs, :])
    ctx.close()


def build_nc(s=S, hpc=HPC, qb_size=512, group_size=2, pair=2, **kwargs):
    nc = bacc.Bacc(
        "TRN2",
        target_bir_lowering=False,
        debug=False,
        num_devices=N_CORES,
    )
    n_chunks = s // 128
    n_qb = s // qb_size
    n_groups = (n_chunks + group_size - 1) // group_size
    n_pairs = n_groups // pair
    qT = nc.dram_tensor("qT", [hpc, D + 1, s], F16, kind="ExternalInput").ap()
    kT = nc.dram_tensor("kT", [hpc, D + 1, s], F16, kind="ExternalInput").ap()
    vres = nc.dram_tensor(
        "vres", [hpc, 128, n_chunks * D], F16, kind="ExternalInput").ap()
    maskT = nc.dram_tensor(
        "maskT", [n_qb, n_pairs, 128, pair * group_size * qb_size], F16,
        kind="ExternalInput").ap()
    outT = nc.dram_tensor(
        "outT", [n_qb, 128, qb_size], F32, kind="ExternalOutput").ap()
    with tile.TileContext(nc) as tc:
        build_kernel_body(tc, qT, kT, vres, maskT, outT, s=s, hpc=hpc,
                          qb_size=qb_size, group_size=group_size, pair=pair,
                          **kwargs)
    nc.compile()
    return nc


_NC_CACHE = {}


def get_nc(**kwargs):
    key = tuple(sorted(kwargs.items()))
    if key not in _NC_CACHE:
        _NC_CACHE[key] = build_nc(**kwargs)
    return _NC_CACHE[key]


def _prep_qkT16(q, k):
    """fp16 staged Q^T (pre-scaled, with B row) and K^T (with ones row) for
    one head: exactly the tensors the device sees."""
    qscale = np.float32(A_CONST * SCALE)
    qT = np.concatenate([q.T * qscale,
                         np.full((1, S), B_CONST, np.float32)],
                        axis=0).astype(np.float16)
    kT = np.concatenate([k.T, np.ones((1, S), np.float32)],
                        axis=0).astype(np.float16)
    return qT, kT


def compute_host_den(query, key, self_attn_mask, qb_size=512, group_size=2,
                     dve_period=DVE_PERIOD, dve_slots=DVE_SLOTS):
    """Replicate the device softmax numerators' per-tile exp formulas and
    reduce over k to get the denominator [B, H, S] (indexed by q)."""
    q = np.asarray(query, dtype=np.float32)
    k = np.asarray(key, dtype=np.float32)
    m = np.asarray(self_attn_mask)
    n_qb = S // qb_size
    n_groups = (S // 128) // group_size
    dset = set(dve_slots)
    den = np.empty((B, H, S), np.float32)
    for b_ in range(B):
        keep = (~m[b_, 0]).T.astype(np.float32)  # [k, q]
        for h_ in range(H):
            flat = b_ * H + h_
            hh = flat % HPC  # position within the core's slot schedule
            qT16, kT16 = _prep_qkT16(q[b_, h_], k[b_, h_])
            # y[k, q] = A*x + B, as accumulated by the PE from fp16 inputs.
            y = kT16.astype(np.float32).T @ qT16.astype(np.float32)
            d_h = np.zeros((S,), np.float32)
            dsum = np.zeros((n_qb, S // 128 // group_size, qb_size),
                            np.float32)
            for qb_ in range(n_qb):
                qs = slice(qb_ * qb_size, (qb_ + 1) * qb_size)
                for gi_ in range(n_groups):
                    si = (qb_ * n_groups + gi_) * HPC + hh
                    ks = slice(gi_ * group_size * 128,
                               (gi_ + 1) * group_size * 128)
                    yt = y[ks, qs]
                    if (si % dve_period) in dset:
                        p = np.rint(yt).astype(np.int16).view(np.float16)
                        p = p.astype(np.float32)
                    else:
                        p = np.exp((yt - B_CONST) / A_CONST)
                    dsum[qb_, gi_] = (keep[ks, qs] * p).sum(axis=0)
            den[b_, h_] = dsum.sum(axis=1).reshape(S)
    return den


def make_in_maps(query, key, value, self_attn_mask, qb_size=512, group_size=2,
                 pair=2):
    """Host-side shard + layout prep. Returns list of 8 per-core input dicts."""
    q = np.asarray(query, dtype=np.float32)
    k = np.asarray(key, dtype=np.float32)
    v = np.asarray(value, dtype=np.float32)
    m = np.asarray(self_attn_mask)
    n_chunks = S // 128
    n_qb = S // qb_size
    n_groups = n_chunks // group_size
    n_pairs = n_groups // pair
    in_maps = []
    # Pre-tiled mask per batch (shared by all cores of that batch):
    # maskM[qb, pr, p, (g, c, q)] = keep[k, q] with
    # k = ((pr*pair + g)*group_size + c)*128 + p.
    maskM = {}
    for b_ in range(B):
        mk = (~m[b_, 0]).T  # [k, q] keep-mask
        t = mk.reshape(n_pairs, pair, group_size, 128, n_qb, qb_size)
        t = t.transpose(4, 0, 3, 1, 2, 5)
        maskM[b_] = np.ascontiguousarray(
            t.reshape(n_qb, n_pairs, 128, pair * group_size * qb_size)
        ).astype(np.float16)
    for core in range(N_CORES):
        flats = [HPC * core + i for i in range(HPC)]
        pairs = [(f // H, f % H) for f in flats]
        b = pairs[0][0]
        qkT = [_prep_qkT16(q[b_, h_], k[b_, h_]) for b_, h_ in pairs]
        qT = np.stack([t[0] for t in qkT])
        kT = np.stack([t[1] for t in qkT])
        # [S,64] -> chunk-major per partition: [128, n_chunks*64] contiguous.
        vres = np.stack([
            v[b_, h_]
            .reshape(n_chunks, 128, D).transpose(1, 0, 2)
            .reshape(128, n_chunks * D)
            for b_, h_ in pairs]).astype(np.float16)
        in_maps.append({
            "qT": np.ascontiguousarray(qT),
            "kT": np.ascontiguousarray(kT),
            "vres": np.ascontiguousarray(vres),
            "maskT": maskM[b],
        })
    return in_maps


def gather_output(results, den):
    out = np.empty((B, H, S, D), np.float32)
    for core, r in enumerate(results):
        oT = r["outT"].astype(np.float32)  # [n_qb, 128, qb_size]
        n_qb = oT.shape[0]
        for i in range(HPC):
            f = HPC * core + i
            b_, h_ = f // H, f % H
            # rows i*64:(i+1)*64 of each q-block hold this head's out^T
            hT = oT[:, i * D:(i + 1) * D, :].transpose(1, 0, 2).reshape(D, S)
            out[b_, h_] = (hT / den[b_, h_][None, :]).T
    return out


def kernel(query, key, value, self_attn_mask, trace=False, tmpdir=None,
           **build_kwargs):
    nc = get_nc(**build_kwargs)
    in_maps = make_in_maps(query, key, value, self_attn_mask)
    kwargs = {"tmpdir": tmpdir} if tmpdir else {}
    res = run_bass_kernel_spmd(nc, in_maps, core_ids=list(range(N_CORES)),
                               trace=trace, **kwargs)
    den = compute_host_den(
        query, key, self_attn_mask,
        dve_period=build_kwargs.get("dve_period", DVE_PERIOD),
        dve_slots=build_kwargs.get("dve_slots", DVE_SLOTS))
    out = gather_output(res.results, den)
    if trace:
        kernel.last_result = res
    return out


# revision 22
# speedup vs baseline: 1.2445x; 1.2445x over previous
"""Masked attention (B=2, H=8, S=4096, D=64) on 8 Trainium2 NeuronCores.

Sharding: batch*head parallel. Core c owns flat heads {2c, 2c+1} (same batch
index b = c // 4 for both, so the [S, S] mask is shared by both heads of a
core).

Device algorithm (per core, per head), transposed so no on-chip transposes are
ever needed; the exp() of the softmax is SPLIT between the ScalarE (true exp),
the VectorE (Schraudolph bit-trick exp), and the GpSimd engine (mask
multiplies) because the 33.5M score elements/core would otherwise bottleneck
any single drain engine behind the PE's ~218us matmul stream:

  - Host supplies K^T augmented with a ones row as [65, S] fp16, Q^T pre-scaled
    by A*SCALE with a B row as [65, S] fp16, so the score matmul directly
    produces y[k, q] = A*x + B in PSUM, where x = (q . k)/sqrt(D) is the true
    logit, A = 1024/ln2 and B = 1024*(15 - c_rms). V is augmented with a ones
    column as [S, 65] fp16 (row 64 of the AV output = softmax denominator) and
    laid out chunk-major per partition so its DMA is contiguous. The mask is
    pre-tiled by the host into contiguous [128, 2048] DMA tiles (one per
    (q-block, pair-of-chunk-groups)) so each mask DMA is a cheap 2D descriptor
    issued from the otherwise-idle Sync queue.
  - Scores are computed transposed: y[k, q] via matmul(lhsT=K^T chunk [65,128],
    rhs=Q^T block [65, 512]); fp16 streams 1 column/cycle and keeps the HAM
    clock at 2.4 GHz.
  - Per score tile (a (2-chunk group, head) pair), one of two drain paths:
      ACT path: ScalarE activation computes pt = exp(y/A - B/A) = exp(x)
        (PSUM->SBUF fp16), then VectorE (or, every gp_tt_every-th ACT tile,
        GpSimd) multiplies by the keep-mask (fp16 2x).
      DVE path (1/3 of tiles): one fused VectorE tensor_mul with int16 output:
        i16 = convert(y * maskT). Bit-cast as fp16 this is Schraudolph's
        approximate exp (rel err ~1.7% RMS), and masked entries are exactly
        0x0000 = 0.0. One 1x-rate pass does drain+mask+exp, freeing ScalarE.
    No max-subtraction is needed: scores ~ N(0,1), exp stays in range.
  - AV accumulates transposed-free: matmul(lhsT=[V|1] chunk [128,65],
    rhs=P^T chunk [128,512], fp16) accumulates out^T[d,q] over the 32 k-chunks
    in PSUM; row 64 = softmax denominator. Each slot's AV matmuls are emitted
    AFTER the next slot's score matmuls (PE executes matmuls strictly in
    order, and AV depends on the softmax chain: emitting AV one slot late
    keeps the score stream ahead of ScalarE/VectorE at q-block boundaries).
  - A finished AV accumulator is flushed (PSUM -> SBUF copy on alternating
    Scalar/Vector, then DMA to DRAM) immediately when its last chunk's AV
    matmul is emitted, so the copy clears the PSUM bank well before the next
    q-block's accumulation needs it. The host divides rows 0:64 by row 64 and
    transposes to [S, 64] during unshard.
"""

from contextlib import ExitStack

import numpy as np

import concourse.tile as tile
from concourse import bacc, mybir
from concourse.bass_utils import run_bass_kernel_spmd

B, H, S, D = 2, 8, 4096, 64
N_CORES = 8
HPC = (B * H) // N_CORES  # heads per core = 2
SCALE = 1.0 / 8.0  # 1/sqrt(D)

# Schraudolph constants for fp16 (10-bit mantissa, bias 15):
#   i16 = round(A*x + B); bitcast(i16) ~= exp(x), rel err ~1.7% RMS.
# A = 1024/ln2; B = 1024*(15 - c) with c ~= 0.0573 (RMS-optimal), rounded so
# B is exactly representable in fp16 (it is sent as a Q^T row).
A_CONST = 1477.3197218702985
B_CONST = 15304.0

F32 = mybir.dt.float32
BF16 = mybir.dt.bfloat16
F16 = mybir.dt.float16
I16 = mybir.dt.int16


def build_kernel_body(tc, qT, kT, vaug, maskT, outT, s=S, hpc=HPC, qb_size=512,
                      group_size=2, psum_s_bufs=3, pt_bufs=8, mask_bufs=8,
                      dve_period=18, dve_slots=(2, 5, 8, 11, 14, 17),
                      gp_tt_every=3, mask_lookahead=3, av_defer=4,
                      warm_mms=16, pair=2):
    """Emit the attention program. All APs are DRAM tensors:
    qT, kT: [hpc, 65, s] f16; vaug: [hpc, 128, n_chunks*65] f16;
    maskT: [n_qb, n_pairs, 128, pair*group_size*qb_size] f16 (pre-tiled);
    outT: [hpc, 65, s] f32.
    """
    nc = tc.nc
    n_qb = s // qb_size
    n_chunks = s // 128
    groups = []
    c0 = 0
    while c0 < n_chunks:
        groups.append((c0, min(group_size, n_chunks - c0)))
        c0 += group_size
    n_pairs = len(groups) // pair

    ctx = ExitStack()
    const = ctx.enter_context(tc.tile_pool(name="const", bufs=1))
    mask_pool = ctx.enter_context(tc.tile_pool(name="mask", bufs=mask_bufs))
    pt_pool = ctx.enter_context(tc.tile_pool(name="pt", bufs=pt_bufs))
    out_pool = ctx.enter_context(tc.tile_pool(name="osb", bufs=4))
    psum_s_pool = ctx.enter_context(
        tc.tile_pool(name="psum_s", bufs=psum_s_bufs, space="PSUM"))
    psum_av_pool = ctx.enter_context(
        tc.tile_pool(name="psum_av", bufs=hpc, space="PSUM"))

    # Resident tensors: Q^T, K^T (fp16, 65 rows: d + affine row), V|1 chunked.
    qT_sb = const.tile([D + 1, hpc, s], F16)
    kT_sb = const.tile([D + 1, hpc, s], F16)
    vaug_sb = const.tile([128, hpc, n_chunks, D + 1], F16)
    # Per-partition bias for the ACT path: exp(y/A - B/A) = exp(x).
    bias_sb = const.tile([128, 1], F32)
    nc.gpsimd.memset(bias_sb[:, :], -B_CONST / A_CONST)
    # Dummy 1-element exp so the ~2.7us ACT_TABLE_LOAD happens during the
    # prologue DMA debt instead of delaying the first real exp.
    tl_sb = const.tile([128, 1], F32)
    nc.scalar.activation(tl_sb[:, :], bias_sb[:, :],
                         mybir.ActivationFunctionType.Exp)

    # Mask tiles stream on the GpSimd software-DGE queue as flat pre-tiled 2D
    # blocks (cheap descriptors; each DMA covers `pair` chunk-groups), so
    # their issue ops and buffer-reuse waits never serialize behind the big
    # const loads or the output DMAs on the Sync queue.
    mask_plan = [(qb_, pr_) for qb_ in range(n_qb) for pr_ in range(n_pairs)]
    mask_tiles = {}
    mask_next = [0]

    def issue_masks(upto_pairs):
        while mask_next[0] < min(upto_pairs, len(mask_plan)):
            qb_, pr_ = mask_plan[mask_next[0]]
            mt = mask_pool.tile([128, pair, group_size, qb_size], F16)
            nc.gpsimd.dma_start(
                out=mt[:, :, :, :],
                in_=maskT[qb_, pr_].rearrange(
                    "p (g c q) -> p g c q", g=pair, c=group_size),
            )
            for g2 in range(pair):
                mask_tiles[(qb_, pr_ * pair + g2)] = mt[:, g2]
            mask_next[0] += 1

    # Prologue DMAs in need-order. Measured queue behavior: both DGE queues
    # deliver nothing before ~7us, then the GpSimd software queue bursts at
    # ~300 GB/s while the Sync hardware queue ramps much slower. So all
    # loads needed in the first ~20us (K^T front, Q^T prefix, V front, early
    # masks) go on the software queue interleaved by first-use time, and the
    # late-need remainders go on Sync.
    vaug_r = [vaug[h, :, :].rearrange("p (c w) -> p c w", w=D + 1)
              for h in range(hpc)]
    vpre = 2 * group_size
    for h in range(hpc):
        nc.gpsimd.dma_start(out=kT_sb[:, h, 0:qb_size],
                            in_=kT[h, :, 0:qb_size])
        nc.gpsimd.dma_start(out=qT_sb[:, h, 0:qb_size],
                            in_=qT[h, :, 0:qb_size])
    issue_masks(1)
    for h in range(hpc):
        nc.gpsimd.dma_start(out=vaug_sb[:, h, 0:vpre, :],
                            in_=vaug_r[h][:, 0:vpre, :])
    for h in range(hpc):
        nc.gpsimd.dma_start(out=kT_sb[:, h, qb_size:2 * qb_size],
                            in_=kT[h, :, qb_size:2 * qb_size])
    issue_masks(2)
    for h in range(hpc):
        nc.gpsimd.dma_start(out=vaug_sb[:, h, vpre:10, :],
                            in_=vaug_r[h][:, vpre:10, :])
    for h in range(hpc):
        nc.gpsimd.dma_start(out=kT_sb[:, h, 2 * qb_size:4 * qb_size],
                            in_=kT[h, :, 2 * qb_size:4 * qb_size])
    issue_masks(3)
    for h in range(hpc):
        nc.gpsimd.dma_start(out=vaug_sb[:, h, 10:16, :],
                            in_=vaug_r[h][:, 10:16, :])
    issue_masks(4)

    # HAM warm-up: fp16 matmuls on a memset tile, needing no DMA — they span
    # the prologue DMA debt and bring the PE clock to 2.4 GHz before the
    # first real score matmul issues.
    warm = const.tile([128, qb_size], F16)
    nc.vector.memset(warm, 0.0)
    wp = psum_s_pool.tile([128, group_size, qb_size], F32, name="wp", tag="ps")
    for _ in range(warm_mms):
        nc.tensor.matmul(wp[:, 0, :], lhsT=warm[:, 0:128], rhs=warm[:, :],
                         start=True, stop=True)

    # Late-need remainders on the Sync hardware queue (first needed ~25us+).
    for h in range(hpc):
        nc.sync.dma_start(out=kT_sb[:, h, 4 * qb_size:6 * qb_size],
                          in_=kT[h, :, 4 * qb_size:6 * qb_size])
    for h in range(hpc):
        nc.sync.dma_start(out=vaug_sb[:, h, 16:24, :],
                          in_=vaug_r[h][:, 16:24, :])
    for h in range(hpc):
        nc.sync.dma_start(out=kT_sb[:, h, 6 * qb_size:],
                          in_=kT[h, :, 6 * qb_size:])
    for h in range(hpc):
        nc.sync.dma_start(out=vaug_sb[:, h, 24:, :],
                          in_=vaug_r[h][:, 24:, :])
    for h in range(hpc):
        nc.sync.dma_start(out=qT_sb[:, h, qb_size:], in_=qT[h, :, qb_size:])

    # Flat slot schedule: (qb, group, head). AV for slot i is emitted during
    # slot i+av_defer, after that slot's score matmuls.
    slots = []
    for qb in range(n_qb):
        for gi, (c0_, gs_) in enumerate(groups):
            for h in range(hpc):
                slots.append((qb, gi, c0_, gs_, h))

    av_cur = {}  # h -> (tile, qb, qs)
    flush_count = [0]

    def flush_av(h):
        # Drain a finished accumulator: PSUM -> SBUF (alternating engines to
        # balance the two near-critical drain engines), then DMA to DRAM.
        avt, _, qs_ = av_cur[h]
        osb = out_pool.tile([D + 1, qb_size], F32, name="osb")
        if flush_count[0] % 2 == 0:
            nc.vector.tensor_copy(osb[:, :], avt[:, :])
        else:
            nc.scalar.copy(osb[:, :], avt[:, :])
        flush_count[0] += 1
        nc.sync.dma_start(out=outT[h, :, qs_], in_=osb[:, :])
        av_cur[h] = None

    def emit_av(qb, c0_, gs_, h, pt, qs):
        if av_cur.get(h) is None:
            avt = psum_av_pool.tile([D + 1, qb_size], F32, tag="av", name="av")
            av_cur[h] = (avt, qb, qs)
        avt = av_cur[h][0]
        for j in range(gs_):
            c = c0_ + j
            nc.tensor.matmul(
                avt[:, :],
                lhsT=vaug_sb[:, h, c, :],
                rhs=pt[:, j, :],
                start=(c == 0),
                stop=(c == n_chunks - 1),
            )
        # Flush as soon as the accumulation over all chunks is fully emitted:
        # the PSUM->SBUF copy then runs several slots before the next q-block
        # needs this PSUM bank back.
        if c0_ + gs_ == n_chunks:
            flush_av(h)

    deferred = []
    pending_tt = []
    act_count = 0
    for si, (qb, gi, c0_, gs_, h) in enumerate(slots):
        qs = slice(qb * qb_size, (qb + 1) * qb_size)
        if h == 0:
            gidx = qb * len(groups) + gi
            issue_masks(gidx // pair + 1 + mask_lookahead)
        mt = mask_tiles[(qb, gi)]

        ps = psum_s_pool.tile([128, group_size, qb_size], F32)
        for j in range(gs_):
            c = c0_ + j
            nc.tensor.matmul(
                ps[:, j, :],
                lhsT=kT_sb[:, h, c * 128:(c + 1) * 128],
                rhs=qT_sb[:, h, qs],
                start=True,
                stop=True,
            )

        pt = pt_pool.tile([128, group_size, qb_size], F16)
        is_dve = (si % dve_period) in dve_slots
        if is_dve:
            # Fused drain+mask+exp on VectorE: i16 = convert(y*mask); the fp16
            # bit pattern of i16 = round(A*x+B) approximates exp(x); mask=0
            # gives exactly 0.0. Emitted ahead of the previous ACT slot's
            # mask-multiply so it runs concurrently with that ACTIVATE on the
            # in-order Vector queue.
            nc.vector.tensor_mul(
                pt[:, :gs_, :].bitcast(I16), ps[:, :gs_, :], mt[:, :gs_, :])
        else:
            nc.scalar.activation(
                pt[:, :gs_, :], ps[:, :gs_, :],
                mybir.ActivationFunctionType.Exp,
                scale=1.0 / A_CONST, bias=bias_sb[:, :],
            )
        # Previous ACT slot's mask-multiply: deferred one slot so this slot's
        # fused DVE drain (if any) sits ahead of it on the Vector queue.
        if pending_tt:
            opt, omt, ogs, use_gp = pending_tt.pop()
            eng = nc.gpsimd if use_gp else nc.vector
            eng.tensor_mul(opt[:, :ogs, :], opt[:, :ogs, :], omt[:, :ogs, :])
        if not is_dve:
            use_gp = bool(gp_tt_every) and (
                act_count % gp_tt_every == gp_tt_every - 1)
            act_count += 1
            pending_tt.append((pt, mt, gs_, use_gp))

        deferred.append((qb, c0_, gs_, h, pt, qs))
        if len(deferred) > av_defer:
            emit_av(*deferred.pop(0))
    if pending_tt:
        opt, omt, ogs, use_gp = pending_tt.pop()
        eng = nc.gpsimd if use_gp else nc.vector
        eng.tensor_mul(opt[:, :ogs, :], opt[:, :ogs, :], omt[:, :ogs, :])
    while deferred:
        emit_av(*deferred.pop(0))
    for h in range(hpc):
        if av_cur.get(h) is not None:
            flush_av(h)
    ctx.close()


def build_nc(s=S, hpc=HPC, qb_size=512, group_size=2, pair=2, **kwargs):
    nc = bacc.Bacc(
        "TRN2",
        target_bir_lowering=False,
        debug=False,
        num_devices=N_CORES,
    )
    n_chunks = s // 128
    n_qb = s // qb_size
    n_groups = (n_chunks + group_size - 1) // group_size
    n_pairs = n_groups // pair
    qT = nc.dram_tensor("qT", [hpc, D + 1, s], F16, kind="ExternalInput").ap()
    kT = nc.dram_tensor("kT", [hpc, D + 1, s], F16, kind="ExternalInput").ap()
    vaug = nc.dram_tensor(
        "vaug", [hpc, 128, n_chunks * (D + 1)], F16, kind="ExternalInput").ap()
    maskT = nc.dram_tensor(
        "maskT", [n_qb, n_pairs, 128, pair * group_size * qb_size], F16,
        kind="ExternalInput").ap()
    outT = nc.dram_tensor("outT", [hpc, D + 1, s], F32, kind="ExternalOutput").ap()
    with tile.TileContext(nc) as tc:
        build_kernel_body(tc, qT, kT, vaug, maskT, outT, s=s, hpc=hpc,
                          qb_size=qb_size, group_size=group_size, pair=pair,
                          **kwargs)
    nc.compile()
    return nc


_NC_CACHE = {}


def get_nc(**kwargs):
    key = tuple(sorted(kwargs.items()))
    if key not in _NC_CACHE:
        _NC_CACHE[key] = build_nc(**kwargs)
    return _NC_CACHE[key]


def make_in_maps(query, key, value, self_attn_mask, qb_size=512, group_size=2,
                 pair=2):
    """Host-side shard + layout prep. Returns list of 8 per-core input dicts."""
    q = np.asarray(query, dtype=np.float32)
    k = np.asarray(key, dtype=np.float32)
    v = np.asarray(value, dtype=np.float32)
    m = np.asarray(self_attn_mask)
    n_chunks = S // 128
    n_qb = S // qb_size
    n_groups = n_chunks // group_size
    n_pairs = n_groups // pair
    in_maps = []
    ones = np.ones((S, 1), np.float32)
    qscale = np.float32(A_CONST * SCALE)
    # Pre-tiled mask per batch (shared by all cores of that batch):
    # maskM[qb, pr, p, (g, c, q)] = keep[k, q] with
    # k = ((pr*pair + g)*group_size + c)*128 + p.
    maskM = {}
    for b_ in range(B):
        mk = (~m[b_, 0]).T  # [k, q] keep-mask
        t = mk.reshape(n_pairs, pair, group_size, 128, n_qb, qb_size)
        t = t.transpose(4, 0, 3, 1, 2, 5)
        maskM[b_] = np.ascontiguousarray(
            t.reshape(n_qb, n_pairs, 128, pair * group_size * qb_size)
        ).astype(np.float16)
    for core in range(N_CORES):
        flats = [HPC * core + i for i in range(HPC)]
        pairs = [(f // H, f % H) for f in flats]
        b = pairs[0][0]
        qT = np.stack([
            np.concatenate([q[b_, h_].T * qscale,
                            np.full((1, S), B_CONST, np.float32)], axis=0)
            for b_, h_ in pairs]).astype(np.float16)
        kT = np.stack([
            np.concatenate([k[b_, h_].T, np.ones((1, S), np.float32)], axis=0)
            for b_, h_ in pairs]).astype(np.float16)
        # [S,65] -> chunk-major per partition: [128, n_chunks*65] contiguous.
        vaug = np.stack([
            np.concatenate([v[b_, h_], ones], axis=1)
            .reshape(n_chunks, 128, D + 1).transpose(1, 0, 2)
            .reshape(128, n_chunks * (D + 1))
            for b_, h_ in pairs]).astype(np.float16)
        in_maps.append({
            "qT": np.ascontiguousarray(qT),
            "kT": np.ascontiguousarray(kT),
            "vaug": np.ascontiguousarray(vaug),
            "maskT": maskM[b],
        })
    return in_maps


def gather_output(results):
    out = np.empty((B, H, S, D), np.float32)
    for core, r in enumerate(results):
        oT = r["outT"].astype(np.float32)  # [HPC, 65, S]
        for i in range(HPC):
            f = HPC * core + i
            b_, h_ = f // H, f % H
            out[b_, h_] = (oT[i, :D, :] / oT[i, D:D + 1, :]).T
    return out


def kernel(query, key, value, self_attn_mask, trace=False, tmpdir=None,
           **build_kwargs):
    nc = get_nc(**build_kwargs)
    in_maps = make_in_maps(query, key, value, self_attn_mask)
    kwargs = {"tmpdir": tmpdir} if tmpdir else {}
    res = run_bass_kernel_spmd(nc, in_maps, core_ids=list(range(N_CORES)),
                               trace=trace, **kwargs)
    out = gather_output(res.results)
    if trace:
        kernel.last_result = res
    return out
